# revision 2
# baseline (speedup 1.0000x reference)
"""Multi-head self-attention on 8 TRN2 NeuronCores — v13.

v12 + algebraic/structural cuts:
  - K-bias dropped entirely: the q-side term (bk . q) is a per-query
    additive constant on scores, which cancels exactly in softmax.
  - V-bias folded into the output-projection bias on the host:
    attention rows sum to 1, so bv contributes proj_w @ bv to every
    output row.
  - V' and K projected in ONE matmul group per k-block (both
    token-major with the same xstage stationary, moving = [wv|wk]);
    K is then transposed feature-major on the PE (fp16 transpose).
    16 matmuls -> 8 + 1 transpose per k-block.
  - Q-bias applied on ACT (Identity activation with fp16 bias AP from
    the blob): the small f32 bias DMA is gone; 2 input DMAs total.
"""

import numpy as np

B, S, D = 4, 2048, 1024
H, DK = 16, 64
SQ = S // 2
SCALE = 64 ** -0.5
NCORES = 8

# blob column offsets (fp16 elements per partition)
_WQ = 0                      # [128, 16h, 8dt, 64]
_WVK = _WQ + 16 * 8 * 64         # [128, 8dt, 16h*128(64v|64k)]
_PW = _WVK + 8 * 2048            # [128(64 used), 16h, 1024]
_ACC = _PW + 16 * 1024           # [128, 8qb, 1024]
_KVK = _ACC + 8 * 1024           # [128, 130] ones col 64
_ZST = _KVK + 130                # [*, 65] zeros (dummy stationary)
_ZMV = _ZST + 65                 # [*, 512] zeros (dummy moving)
_ID = _ZMV + 512                 # [128, 128] identity
_BQ = _ID + 128                  # [128(64 used), 16] q-bias fp16
_NBLOB = _BQ + 16

_cache = {}
LAST_EXEC_TIME_NS = None


def _build_nc(repeat=1):
    import concourse.bass as bass
    import concourse.mybir as mybir
    import concourse.tile as tile
    from concourse import bacc
    from concourse.bass import ds, ts

    fp16 = mybir.dt.float16
    f32 = mybir.dt.float32
    mult = mybir.AluOpType.mult
    add = mybir.AluOpType.add

    nc = bacc.Bacc(target_bir_lowering=False, debug=False, num_devices=NCORES)

    blob_d = nc.dram_tensor("blob", [128, _NBLOB], fp16, kind="ExternalInput")
    bqk_d = nc.dram_tensor("bqk", [128, 16], f32, kind="ExternalInput")
    xtq_d = nc.dram_tensor("xtq", [128, 8, S + SQ], fp16, kind="ExternalInput")
    out_d = nc.dram_tensor("out", [SQ, D], fp16, kind="ExternalOutput")

    with tile.TileContext(nc) as tc:
        with (
            tc.tile_pool(name="const", bufs=1) as const,
            tc.tile_pool(name="work", bufs=1) as work,
            tc.tile_pool(name="psA", bufs=1, space="PSUM") as psA,
            tc.tile_pool(name="psB", bufs=1, space="PSUM") as psB,
            tc.tile_pool(name="psC", bufs=1, space="PSUM") as psC,
            tc.tile_pool(name="psD", bufs=1, space="PSUM") as psD,
        ):
            blob = const.tile([128, _NBLOB], fp16, tag="blob", name="blob")
            nc.sync.dma_start(out=blob, in_=blob_d.ap())
            bqk = const.tile([128, 16], f32, tag="bqk", name="bqk")
            nc.sync.dma_start(out=bqk, in_=bqk_d.ap())

            wq_all = blob[:, _WQ:_WVK].rearrange(
                "p (h d c) -> p h d c", h=16, d=8)
            wvk = blob[:, _WVK:_PW].rearrange("p (d f) -> p d f", d=8)
            pw = blob[:, _PW:_ACC].rearrange("p (h f) -> p h f", h=16)
            acc = blob[:, _ACC:_KVK].rearrange("p (q f) -> p q f", q=8)
            kvk = blob[:, _KVK:_ZST]
            zst = blob[0:1, _ZST:_ZMV]
            zmv = blob[0:1, _ZMV:_ID]
            ident = blob[:, _ID:_BQ]
            bq = blob[:, _BQ:_NBLOB]

            def body():
                xtq = work.tile([128, 8, S + SQ], fp16, tag="xtq", name="xtq")
                nc.sync.dma_start(out=xtq, in_=xtq_d.ap())

                wqstage = work.tile([128, 8, 64], fp16, tag="wqstage",
                                    name="wqstage")
                xstage = work.tile([128, 8, 128], fp16, tag="xstage",
                                   name="xstage")
                qt = work.tile([64, SQ], fp16, tag="qt", name="qt")
                kstage = work.tile([64, 128], fp16, tag="kstage", name="kstage")
                e = work.tile([128, SQ], fp16, tag="e", name="e")
                otn = work.tile([64, SQ], fp16, tag="otn", name="otn")
                rec = work.tile([1, SQ], fp16, tag="rec", name="rec")
                recb = work.tile([64, SQ], fp16, tag="recb", name="recb")

                with tc.For_i(0, 16) as h:
                    nc.scalar.copy(wqstage, wq_all[:, ds(h, 1), :, :])

                    # Q projection + bias: qt[qf, q] for this head
                    psq = psA.tile([64, SQ], f32, tag="pq", name="psq")
                    for dt in range(8):
                        for o in (0, 512):
                            nc.tensor.matmul(psq[:, o:o + 512],
                                             wqstage[:, dt, :],
                                             xtq[:, dt, S + o:S + o + 512],
                                             start=(dt == 0), stop=(dt == 7))
                    nc.vector.tensor_scalar(qt[:], psq, bqk[0:64, ds(h, 1)],
                                            None, add)

                    # zero the AV accumulator rows [0:65]
                    ot = psC.tile([128, SQ], f32, tag="ot", name="ot")
                    for o in (0, 512):
                        nc.tensor.matmul(ot[0:65, o:o + 512], zst, zmv,
                                         start=True, stop=False,
                                         skip_group_check=True)

                    with tc.For_i(0, 16) as kt:
                        nc.scalar.copy(xstage, xtq[:, :, ts(kt, 128)])
                        # [V' | K] for this (h, kt): [128 tok, 64v|64k]
                        psvk = psD.tile([128, 128], f32, tag="pvk",
                                        name="psvk")
                        for dt in range(8):
                            nc.tensor.matmul(psvk, xstage[:, dt, :],
                                             wvk[:, dt, ts(h, 128)],
                                             start=(dt == 0), stop=(dt == 7))
                        # kvk cols {0:64, 65:129} <- psvk (ones col 64 preset)
                        nc.scalar.copy(
                            kvk.rearrange("p (a b) -> p a b", a=2)[:, :, 0:64],
                            psvk.rearrange("p (a b) -> p a b", a=2))
                        # K^T via PE transpose, then to SBUF
                        psT = psD.tile([64, 128], fp16, tag="pT", name="psT")
                        nc.tensor.transpose(psT, kvk[:, 65:129], ident)
                        nc.scalar.copy(kstage, psT)
                        # scores^T [ktok, q] and exp
                        sc = psB.tile([128, SQ], f32, tag="sc", name="sc")
                        for o in (0, 512):
                            nc.tensor.matmul(sc[:, o:o + 512], kstage,
                                             qt[:, o:o + 512],
                                             start=True, stop=True)
                        nc.scalar.activation(e[:], sc,
                                             mybir.ActivationFunctionType.Exp,
                                             scale=float(SCALE))
                        # AV (+ denominator via ones column)
                        for o in (0, 512):
                            nc.tensor.matmul(ot[0:65, o:o + 512],
                                             kvk[:, 0:65], e[:, o:o + 512],
                                             start=False, stop=False,
                                             skip_group_check=True)

                    with nc.allow_low_precision(reason="fp16 softmax denom"):
                        nc.vector.reciprocal(rec, ot[64:65, :])
                    nc.gpsimd.partition_broadcast(recb, rec)
                    nc.vector.tensor_tensor(otn[:], ot[0:64, :], recb, mult)

                    # folded output projection: acc[:, qb, :] += otn_qb^T @ pw_h
                    for qb in range(8):
                        pspr = psA.tile([128, 1024], f32, tag="pq",
                                        name=f"pspr{qb}")
                        for o in (0, 512):
                            nc.tensor.matmul(pspr[:, o:o + 512],
                                             otn[:, qb * 128:(qb + 1) * 128],
                                             pw[0:64, ds(h, 1), o:o + 512],
                                             start=True, stop=True)
                        nc.vector.tensor_tensor(acc[:, qb, :], acc[:, qb, :],
                                                pspr, add)

                out_ap = bass.AP(tensor=out_d, offset=0,
                                 ap=[[1024, 128], [131072, 8], [1, 1024]])
                nc.sync.dma_start(out=out_ap, in_=acc)

            for _rep in range(repeat):
                body()

    nc.compile()
    return nc


def _prep_shared(qkv_w, qkv_b, proj_w, proj_b):
    f16 = np.float16
    blob = np.zeros((128, _NBLOB), f16)
    wqT = qkv_w[0:1024].T          # [D, 1024]
    wkT = qkv_w[1024:2048].T
    wvT = qkv_w[2048:3072].T
    # wq_all[p, h, dt, c] = wqT[dt*128+p, h*64+c]
    blob[:, _WQ:_WVK] = wqT.reshape(8, 128, 16, 64).transpose(
        1, 2, 0, 3).reshape(128, -1).astype(f16)
    # wvk[p, dt, h*128 + (0:64 -> v, 64:128 -> k)]
    wvp = wvT.reshape(8, 128, 16, 64)    # [dt, p, h, 64]
    wkp = wkT.reshape(8, 128, 16, 64)
    wvk = np.concatenate([wvp, wkp], axis=3)   # [dt, p, h, 128]
    blob[:, _WVK:_PW] = wvk.transpose(1, 0, 2, 3).reshape(128, -1).astype(f16)
    # pw[vf, h, d] = proj_w[d, h*64+vf]
    pwl = proj_w.T.reshape(16, 64, 1024).transpose(1, 0, 2)  # [64, 16, 1024]
    blob[0:64, _PW:_ACC] = pwl.reshape(64, -1).astype(f16)
    # acc init: proj bias with host-folded V-bias (attn rows sum to 1)
    pb_eff = proj_b + proj_w @ qkv_b[2048:3072]
    blob[:, _ACC:_KVK] = np.tile(pb_eff.astype(f16), (128, 8))
    blob[:, _KVK + 64] = 1.0
    blob[:, _ID:_BQ] = np.eye(128, dtype=f16)
    bqk = np.zeros((128, 16), np.float32)
    bqk[0:64] = qkv_b[0:1024].reshape(16, 64).T
    return dict(blob=blob, bqk=bqk)


def _make_in_maps(x, qkv_w, qkv_b, proj_w, proj_b):
    x = np.asarray(x, np.float32)
    shared = _prep_shared(np.asarray(qkv_w, np.float32),
                          np.asarray(qkv_b, np.float32),
                          np.asarray(proj_w, np.float32),
                          np.asarray(proj_b, np.float32))
    in_maps = []
    for c in range(NCORES):
        b, half = c // 2, c % 2
        xT = np.ascontiguousarray(x[b].T).astype(np.float16)   # [D, S]
        xtq = np.empty((128, 8, S + SQ), np.float16)
        xtq[:, :, 0:S] = xT.reshape(8, 128, S).transpose(1, 0, 2)
        xtq[:, :, S:] = xT[:, half * SQ:(half + 1) * SQ].reshape(
            8, 128, SQ).transpose(1, 0, 2)
        m = dict(shared)
        m["xtq"] = xtq
        in_maps.append(m)
    return in_maps


def kernel(x, qkv_w, qkv_b, proj_w, proj_b):
    global LAST_EXEC_TIME_NS
    from concourse.bass_utils import run_bass_kernel_spmd

    in_maps = _make_in_maps(x, qkv_w, qkv_b, proj_w, proj_b)
    if "nc" not in _cache:
        _cache["nc"] = _build_nc()
    nc = _cache["nc"]

    res = run_bass_kernel_spmd(nc, in_maps, core_ids=list(range(NCORES)))
    LAST_EXEC_TIME_NS = res.exec_time_ns

    out = np.zeros((B, S, D), np.float32)
    for c in range(NCORES):
        b, half = c // 2, c % 2
        out[b, half * SQ:(half + 1) * SQ, :] = res.results[c]["out"].astype(
            np.float32)
    return out


# revision 5
# speedup vs baseline: 2.0345x; 2.0345x over previous
"""Multi-head self-attention on 8 TRN2 NeuronCores — v18 (final).

Per-unique-instruction dispatch (~30-50us, serialized) dominates this
stack; hardware-loop re-executions are nearly free. All biases are
eliminated from the device inner loops: K-bias cancels in softmax,
V-bias is host-folded into the output bias, Q-bias rides the VK
projection as an extra moving column (bsc) applied via the Exp
activation's per-partition bias AP. Two input DMAs total.

v12 + algebraic/structural cuts:
  - K-bias dropped entirely: the q-side term (bk . q) is a per-query
    additive constant on scores, which cancels exactly in softmax.
  - V-bias folded into the output-projection bias on the host:
    attention rows sum to 1, so bv contributes proj_w @ bv to every
    output row.
  - V' and K projected in ONE matmul group per k-block (both
    token-major with the same xstage stationary, moving = [wv|wk]);
    K is then transposed feature-major on the PE (fp16 transpose).
    16 matmuls -> 8 + 1 transpose per k-block.
  - Q-bias applied on ACT (Identity activation with fp16 bias AP from
    the blob): the small f32 bias DMA is gone; 2 input DMAs total.
"""

import numpy as np

B, S, D = 4, 2048, 1024
H, DK = 16, 64
SQ = S // 2
SCALE = 64 ** -0.5
NCORES = 8

# blob column offsets (fp16 elements per partition)
_HB = 0                    # [128, 16h, 512 wq | 1040 wvkb]
_PW = _HB + 16 * 1552            # [128(64 used), 16h, 1024]
_ACC = _PW + 16 * 1024           # [128, 8qb, 1024]
_KVK = _ACC + 8 * 1024           # [128, 131] ones col 64, bsc col 130
_ZST = _KVK + 131                # [*, 65] zeros (dummy stationary)
_ZMV = _ZST + 65                 # [*, 512] zeros (dummy moving)
_ID = _ZMV + 512                 # [128, 128] identity
_BQ = _ID + 128                  # [128(64 used), 16] q-bias fp16
_ZB = _BQ + 16                   # [128, 1024] zeros (ot init)
_NBLOB = _ZB + 1024

_cache = {}
LAST_EXEC_TIME_NS = None


def _build_nc(repeat=1):
    import concourse.bass as bass
    import concourse.mybir as mybir
    import concourse.tile as tile
    from concourse import bacc
    from concourse.bass import ds, ts

    fp16 = mybir.dt.float16
    f32 = mybir.dt.float32
    mult = mybir.AluOpType.mult
    add = mybir.AluOpType.add

    nc = bacc.Bacc(target_bir_lowering=False, debug=False, num_devices=NCORES)

    blob_d = nc.dram_tensor("blob", [128, _NBLOB], fp16, kind="ExternalInput")
    xtq_d = nc.dram_tensor("xtq", [128, 8, S + SQ], fp16, kind="ExternalInput")
    out_d = nc.dram_tensor("out", [SQ, D], fp16, kind="ExternalOutput")

    with tile.TileContext(nc) as tc:
        with (
            tc.tile_pool(name="const", bufs=1) as const,
            tc.tile_pool(name="work", bufs=1) as work,
            tc.tile_pool(name="psA", bufs=1, space="PSUM") as psA,
            tc.tile_pool(name="psC", bufs=1, space="PSUM") as psC,
            tc.tile_pool(name="psD", bufs=1, space="PSUM") as psD,
        ):
            blob = const.tile([128, _NBLOB], fp16, tag="blob", name="blob")
            nc.sync.dma_start(out=blob, in_=blob_d.ap())

            hb = blob[:, _HB:_PW].rearrange("p (h x) -> p h x", h=16)
            pw = blob[:, _PW:_ACC].rearrange("p (h f) -> p h f", h=16)
            acc = blob[:, _ACC:_KVK].rearrange("p (q f) -> p q f", q=8)
            kvk = blob[:, _KVK:_ZST]
            zst = blob[0:1, _ZST:_ZMV]
            zmv = blob[0:1, _ZMV:_ID]
            ident = blob[:, _ID:_BQ]
            zb = blob[:, _ZB:_NBLOB]

            def body():
                xtq = work.tile([128, 8, S + SQ], fp16, tag="xtq", name="xtq")
                nc.sync.dma_start(out=xtq, in_=xtq_d.ap())

                wstage = work.tile([128, 1552], fp16, tag="wstage",
                                   name="wstage")
                xstage = work.tile([128, 8, 128], fp16, tag="xstage",
                                   name="xstage")
                qt = work.tile([64, SQ], fp16, tag="qt", name="qt")
                kstage = work.tile([64, 128], fp16, tag="kstage", name="kstage")
                e = work.tile([128, SQ], fp16, tag="e", name="e")
                otn = work.tile([64, SQ], fp16, tag="otn", name="otn")
                rec = work.tile([1, SQ], fp16, tag="rec", name="rec")
                recb = work.tile([64, SQ], fp16, tag="recb", name="recb")

                with tc.For_i(0, 16) as h:
                    nc.scalar.copy(wstage, hb[:, ds(h, 1), :])

                    # Q projection + bias: qt[qf, q] for this head
                    psq = psA.tile([64, SQ], f32, tag="pq", name="psq")
                    for dt in range(8):
                        for o in (0, 512):
                            nc.tensor.matmul(psq[:, o:o + 512],
                                             wstage[:, dt * 64:(dt + 1) * 64],
                                             xtq[:, dt, S + o:S + o + 512],
                                             start=(dt == 0), stop=(dt == 7))
                    nc.scalar.copy(qt, psq)

                    # zero the AV accumulator rows [0:65] (ACT copy of zeros)
                    ot = psC.tile([128, SQ], f32, tag="ot", name="ot")
                    nc.scalar.copy(ot[0:65, :], zb[0:65, :])

                    with tc.For_i(0, 16) as kt:
                        nc.scalar.copy(xstage, xtq[:, :, ts(kt, 128)])
                        # [V' | K] for this (h, kt): [128 tok, 64v|64k]
                        psvk = psD.tile([128, 129], f32, tag="pvk",
                                        name="psvk")
                        for dt in range(8):
                            nc.tensor.matmul(
                                psvk, xstage[:, dt, :],
                                wstage[:, 512 + dt * 130:512 + dt * 130 + 129],
                                start=(dt == 0), stop=(dt == 7))
                        # kvk cols {0:64, 65:129} <- psvk (ones col 64 preset)
                        nc.scalar.copy(
                            kvk[:, 0:130].rearrange(
                                "p (a b) -> p a b", a=2)[:, :, 0:64],
                            psvk[:, 0:128].rearrange(
                                "p (a b) -> p a b", a=2))
                        nc.scalar.copy(kvk[:, 130:131], psvk[:, 128:129])
                        # K^T via PE transpose, then to SBUF
                        psT = psD.tile([64, 128], fp16, tag="pT", name="psT")
                        nc.tensor.transpose(psT, kvk[:, 65:129], ident)
                        nc.scalar.copy(kstage, psT)
                        # scores^T [ktok, q] and exp
                        sc = psA.tile([128, SQ], f32, tag="pq", name="sc")
                        for o in (0, 512):
                            nc.tensor.matmul(sc[:, o:o + 512], kstage,
                                             qt[:, o:o + 512],
                                             start=True, stop=True)
                        nc.scalar.activation(e[:], sc,
                                             mybir.ActivationFunctionType.Exp,
                                             bias=kvk[:, 130:131],
                                             scale=float(SCALE))
                        # AV (+ denominator via ones column)
                        for o in (0, 512):
                            nc.tensor.matmul(ot[0:65, o:o + 512],
                                             kvk[:, 0:65], e[:, o:o + 512],
                                             start=False, stop=False,
                                             skip_group_check=True)

                    with nc.allow_low_precision(reason="fp16 softmax denom"):
                        nc.vector.reciprocal(rec, ot[64:65, :])
                    nc.gpsimd.partition_broadcast(recb, rec)
                    nc.vector.tensor_tensor(otn[:], ot[0:64, :], recb, mult)

                    # folded output projection, two q-blocks per PSUM round:
                    # acc[:, qb:qb+2, :] += otn_{qb,qb+1}^T @ pw_h
                    for qp in range(4):
                        pspr = psA.tile([128, 2, 1024], f32, tag="pq",
                                        name=f"pspr{qp}")
                        for j in range(2):
                            qb = qp * 2 + j
                            for o in (0, 512):
                                nc.tensor.matmul(
                                    pspr[:, j, o:o + 512],
                                    otn[:, qb * 128:(qb + 1) * 128],
                                    pw[0:64, ds(h, 1), o:o + 512],
                                    start=True, stop=True)
                        nc.vector.tensor_tensor(acc[:, qp * 2:qp * 2 + 2, :],
                                                acc[:, qp * 2:qp * 2 + 2, :],
                                                pspr, add)

                out_ap = bass.AP(tensor=out_d, offset=0,
                                 ap=[[1024, 128], [131072, 8], [1, 1024]])
                nc.sync.dma_start(out=out_ap, in_=acc)

            for _rep in range(repeat):
                body()

    nc.compile()
    return nc


def _prep_shared(qkv_w, qkv_b, proj_w, proj_b):
    f16 = np.float16
    blob = np.zeros((128, _NBLOB), f16)
    wqT = qkv_w[0:1024].T          # [D, 1024]
    wkT = qkv_w[1024:2048].T
    wvT = qkv_w[2048:3072].T
    hbv = blob[:, _HB:_PW].reshape(128, 16, 1552)
    hbv[:, :, 0:512] = wqT.reshape(8, 128, 16, 64).transpose(
        1, 2, 0, 3).reshape(128, 16, 512).astype(f16)
    wvp = wvT.reshape(8, 128, 16, 64)    # [dt, p, h, 64]
    wkp = wkT.reshape(8, 128, 16, 64)
    bqh = qkv_b[0:1024].reshape(16, 64)
    wk_rows = qkv_w[1024:2048].reshape(16, 64, 1024)   # [h, dk, d]
    wbs = SCALE * np.einsum('hkd,hk->dh', wk_rows, bqh)   # [1024, 16]
    wbsp = wbs.reshape(8, 128, 16)[:, :, :, None]      # [dt, p, h, 1]
    pad = np.zeros((8, 128, 16, 1), np.float32)
    wvkb = np.concatenate([wvp, wkp, wbsp, pad], axis=3)  # [dt, p, h, 130]
    hbv[:, :, 512:1552] = wvkb.transpose(1, 2, 0, 3).reshape(
        128, 16, 1040).astype(f16)
    # pw[vf, h, d] = proj_w[d, h*64+vf]
    pwl = proj_w.T.reshape(16, 64, 1024).transpose(1, 0, 2)  # [64, 16, 1024]
    blob[0:64, _PW:_ACC] = pwl.reshape(64, -1).astype(f16)
    # acc init: proj bias with host-folded V-bias (attn rows sum to 1)
    pb_eff = proj_b + proj_w @ qkv_b[2048:3072]
    blob[:, _ACC:_KVK] = np.tile(pb_eff.astype(f16), (128, 8))
    blob[:, _KVK + 64] = 1.0
    blob[:, _ID:_BQ] = np.eye(128, dtype=f16)
    return dict(blob=blob)


def _make_in_maps(x, qkv_w, qkv_b, proj_w, proj_b):
    x = np.asarray(x, np.float32)
    shared = _prep_shared(np.asarray(qkv_w, np.float32),
                          np.asarray(qkv_b, np.float32),
                          np.asarray(proj_w, np.float32),
                          np.asarray(proj_b, np.float32))
    in_maps = []
    for c in range(NCORES):
        b, half = c // 2, c % 2
        xT = np.ascontiguousarray(x[b].T).astype(np.float16)   # [D, S]
        xtq = np.empty((128, 8, S + SQ), np.float16)
        xtq[:, :, 0:S] = xT.reshape(8, 128, S).transpose(1, 0, 2)
        xtq[:, :, S:] = xT[:, half * SQ:(half + 1) * SQ].reshape(
            8, 128, SQ).transpose(1, 0, 2)
        m = dict(shared)
        m["xtq"] = xtq
        in_maps.append(m)
    return in_maps


def kernel(x, qkv_w, qkv_b, proj_w, proj_b):
    global LAST_EXEC_TIME_NS
    from concourse.bass_utils import run_bass_kernel_spmd

    in_maps = _make_in_maps(x, qkv_w, qkv_b, proj_w, proj_b)
    if "nc" not in _cache:
        _cache["nc"] = _build_nc()
    nc = _cache["nc"]

    res = run_bass_kernel_spmd(nc, in_maps, core_ids=list(range(NCORES)))
    LAST_EXEC_TIME_NS = res.exec_time_ns

    out = np.zeros((B, S, D), np.float32)
    for c in range(NCORES):
        b, half = c // 2, c % 2
        out[b, half * SQ:(half + 1) * SQ, :] = res.results[c]["out"].astype(
            np.float32)
    return out
